# revision 1
# baseline (speedup 1.0000x reference)
"""KD feature-level smooth-L1 loss kernel for Trainium2 (8 NeuronCores).

Math (per batch sample b over (C,H,W) = 256*64*64 = N elements):
  t_norm = (t - mean) * rsqrt(var + eps)          # LayerNorm, no affine
  d   = |t_norm - s|
  kd  = where(d <= 2, d*d/4, d - 1)               # smooth-L1, beta=2
  out = mean_b( sum_chw(kd) )

Device-side decomposition (per sample, rs = 1/sqrt(var+eps), std = 1/rs):
  x  = t - (s*std + mean)        ->  d = rs*|x|
  dt = |x|                       ->  S_d  = sum(dt)     (ACT Abs + accum)
  mt = min(dt, 2*std)            ->  S_m  = sum(mt)     (DVE TS-min + accum)
  S_q = sum(mt^2)                                      (ACT Square + accum)
  sum(kd) = rs*(S_d - S_m) + 0.25*rs^2*S_q
Sharding: pure data parallel, 4 samples per core; host sums 8 partial
outputs and divides by 32.
"""

import os
from contextlib import ExitStack

import numpy as np

import concourse.bass as bass
import concourse.mybir as mybir
import concourse.tile as tile
from concourse import bacc
from concourse.bass_utils import run_bass_kernel_spmd

B, C, H, W = 32, 256, 64, 64
N_CORES = 8
BPC = B // N_CORES            # samples per core
P = 128
N = C * H * W                 # 1048576 elements per sample
FD = N // P                   # 8192 free-dim per partition
NCH = 4                       # loss chunks per sample
CH = FD // NCH                # 2048
EPS = 1e-5
BETA = 2.0
LOSS_WEIGHT = 1.0

f32 = mybir.dt.float32
AF = mybir.ActivationFunctionType
OP = mybir.AluOpType
AX = mybir.AxisListType


def _build_kernel(ctx: ExitStack, tc: "tile.TileContext", out_ap, teacher, stu):
    nc = tc.nc

    const_pool = ctx.enter_context(tc.tile_pool(name="const", bufs=1))
    t_pool = ctx.enter_context(tc.tile_pool(name="t", bufs=2))
    s_pool = ctx.enter_context(tc.tile_pool(name="s", bufs=2))
    v_pool = ctx.enter_context(tc.tile_pool(name="v", bufs=2))
    x_pool = ctx.enter_context(tc.tile_pool(name="x", bufs=2))
    d_pool = ctx.enter_context(tc.tile_pool(name="d", bufs=2))
    m_pool = ctx.enter_context(tc.tile_pool(name="m", bufs=2))
    dead_pool = ctx.enter_context(tc.tile_pool(name="dead", bufs=2))
    sums_pool = ctx.enter_context(tc.tile_pool(name="sums", bufs=2))
    tiny_pool = ctx.enter_context(tc.tile_pool(name="tiny", bufs=2))
    ps_sumt_pool = ctx.enter_context(tc.tile_pool(name="ps_sumt", bufs=2, space="PSUM"))
    ps_misc_pool = ctx.enter_context(tc.tile_pool(name="ps_misc", bufs=2, space="PSUM"))

    ones = const_pool.tile([P, 1], f32)
    nc.vector.memset(ones[:], 1.0)
    staging = const_pool.tile([1, 16 * BPC], f32)
    nc.vector.memset(staging[:], 0.0)

    for b in range(BPC):
        # ---------------- load teacher sample ----------------
        t_sb = t_pool.tile([P, FD], f32)
        nc.sync.dma_start(t_sb[:, 0 : FD // 2], teacher[b, :, 0 : FD // 2])
        nc.sync.dma_start(t_sb[:, FD // 2 : FD], teacher[b, :, FD // 2 : FD])

        # issue student loads early so they overlap the stats phase
        s_tiles = []
        for i in range(2):
            s_sb = s_pool.tile([P, FD // 2], f32)
            nc.sync.dma_start(s_sb[:], stu[b, :, i * (FD // 2) : (i + 1) * (FD // 2)])
            s_tiles.append(s_sb)

        # per-sample per-partition partial sums:
        # cols 0:4  sum|x| per chunk   4:8  sum(mt)   8:12 sum(mt^2)  12:16 sum(t^2)
        sums = sums_pool.tile([P, 16], f32)

        # ---------------- stats: S_t (PE), S_tt (DVE TTR) ----------------
        ps_t = ps_sumt_pool.tile([1, 512], f32)
        nmm = FD // 512
        for k in range(nmm):
            nc.tensor.matmul(
                ps_t[:, :],
                ones[:, :],
                t_sb[:, k * 512 : (k + 1) * 512],
                start=(k == 0),
                stop=(k == nmm - 1),
            )
        for c in range(NCH):
            sl = slice(c * CH, (c + 1) * CH)
            ttdead = dead_pool.tile([P, CH], f32)
            nc.vector.scalar_tensor_tensor(
                ttdead[:],
                t_sb[:, sl],
                1.0,
                t_sb[:, sl],
                op0=OP.mult,
                op1=OP.mult,
                accum_out=sums[:, 12 + c : 13 + c],
            )

        ps_m = ps_misc_pool.tile([1, 16], f32)
        nc.tensor.matmul(ps_m[:, 12:16], ones[:, :], sums[:, 12:16], start=True, stop=True)

        # ---------------- tiny scalar math ----------------
        # bb cols: 0=std 1=mean 2=thr 3..11 scratch
        bb = tiny_pool.tile([1, 16], f32)
        st = bb[0:1, 3:4]
        nc.vector.reduce_sum(out=st, in_=ps_t[:, :], axis=AX.X)
        stt = bb[0:1, 4:5]
        nc.vector.reduce_sum(out=stt, in_=ps_m[0:1, 12:16], axis=AX.X)
        mean = bb[0:1, 1:2]
        nc.vector.tensor_scalar(mean, st, 1.0 / N, None, op0=OP.mult)
        ve_a = bb[0:1, 5:6]
        nc.vector.tensor_scalar(ve_a, stt, 1.0 / N, EPS, op0=OP.mult, op1=OP.add)
        msq = bb[0:1, 6:7]
        nc.vector.tensor_tensor(msq, mean, mean, op=OP.mult)
        ve = bb[0:1, 7:8]
        nc.vector.tensor_tensor(ve, ve_a, msq, op=OP.subtract)
        inv_ve = bb[0:1, 8:9]
        nc.vector.reciprocal(inv_ve, ve)
        rs = bb[0:1, 9:10]
        nc.scalar.activation(rs, inv_ve, AF.Sqrt)  # rs0 ~= 1/sqrt(ve) (table)
        # two Newton iterations: rs <- rs*(1.5 - 0.5*ve*rs^2)
        for it in range(2):
            r2 = bb[0:1, 10:11]
            nc.vector.tensor_tensor(r2, rs, rs, op=OP.mult)
            pv = bb[0:1, 11:12]
            nc.vector.tensor_tensor(pv, r2, ve, op=OP.mult)
            hh = bb[0:1, 12:13]
            nc.vector.tensor_scalar(hh, pv, -0.5, 1.5, op0=OP.mult, op1=OP.add)
            rs_new = bb[0:1, 13 + it : 14 + it]
            nc.vector.tensor_tensor(rs_new, rs, hh, op=OP.mult)
            rs = rs_new
        stdv = bb[0:1, 0:1]
        nc.vector.tensor_tensor(stdv, ve, rs, op=OP.mult)  # std = ve*rs = sqrt(ve)
        thr = bb[0:1, 2:3]
        nc.vector.tensor_scalar(thr, stdv, BETA, None, op0=OP.mult)

        bcast = tiny_pool.tile([P, 3], f32)
        nc.gpsimd.partition_broadcast(bcast[:, 0:3], bb[0:1, 0:3])
        std_vec = bcast[:, 0:1]
        mean_vec = bcast[:, 1:2]
        thr_vec = bcast[:, 2:3]

        # ---------------- loss passes ----------------
        for c in range(NCH):
            tsl = slice(c * CH, (c + 1) * CH)
            ssb = s_tiles[c // 2]
            ssl = slice((c % 2) * CH, (c % 2 + 1) * CH)

            v = v_pool.tile([P, CH], f32)
            nc.scalar.activation(v[:], ssb[:, ssl], AF.Identity, bias=mean_vec, scale=std_vec)
            x = x_pool.tile([P, CH], f32)
            nc.vector.tensor_tensor(x[:], t_sb[:, tsl], v[:], op=OP.subtract)
            d = d_pool.tile([P, CH], f32)
            nc.scalar.activation(d[:], x[:], AF.Abs, accum_out=sums[:, c : c + 1])
            m = m_pool.tile([P, CH], f32)
            nc.vector.tensor_scalar(
                m[:],
                d[:],
                thr_vec,
                0.0,
                op0=OP.min,
                op1=OP.add,
                accum_out=sums[:, 4 + c : 5 + c],
            )
            # dead output written over x (x is dead after Abs)
            nc.scalar.activation(x[:], m[:], AF.Square, accum_out=sums[:, 8 + c : 9 + c])

        # partition-reduce the 12 loss partials in one matmul
        nc.tensor.matmul(ps_m[:, 0:12], ones[:, :], sums[:, 0:12], start=True, stop=True)
        nc.vector.tensor_copy(staging[0:1, 16 * b : 16 * b + 12], ps_m[0:1, 0:12])
        nc.vector.tensor_copy(staging[0:1, 16 * b + 12 : 16 * b + 13], rs)
        nc.vector.tensor_copy(staging[0:1, 16 * b + 13 : 16 * b + 14], stdv)
        nc.vector.tensor_copy(staging[0:1, 16 * b + 14 : 16 * b + 15], bb[0:1, 1:2])
        nc.vector.tensor_copy(staging[0:1, 16 * b + 15 : 16 * b + 16], ve)

    nc.sync.dma_start(out_ap[:, :], staging[:, :])


_CACHED = {}


def _get_nc():
    if "nc" in _CACHED:
        return _CACHED["nc"]
    nc = bacc.Bacc(
        "TRN2",
        target_bir_lowering=False,
        debug=False,
        enable_asserts=False,
        num_devices=N_CORES,
    )
    teacher = nc.dram_tensor("teacher", [BPC, P, FD], f32, kind="ExternalInput").ap()
    stu = nc.dram_tensor("stu", [BPC, P, FD], f32, kind="ExternalInput").ap()
    out = nc.dram_tensor("out", [1, 16 * BPC], f32, kind="ExternalOutput").ap()
    with tile.TileContext(nc) as tc:
        with ExitStack() as ctx:
            _build_kernel(ctx, tc, out, teacher, stu)
    nc.compile()
    _CACHED["nc"] = nc
    return nc


def _combine(parts):
    """parts: list of 8 arrays [1, 16*BPC] -> scalar loss (float64 math)."""
    losses = []
    for r in parts:
        r = np.asarray(r, dtype=np.float64).reshape(BPC, 16)
        S_d = r[:, 0:4].sum(axis=1)
        S_m = r[:, 4:8].sum(axis=1)
        S_q = r[:, 8:12].sum(axis=1)
        rs = r[:, 12]
        losses.append(rs * (S_d - S_m) + 0.25 * rs * rs * S_q)
    losses = np.concatenate(losses)
    return np.float32(LOSS_WEIGHT * losses.mean())


def run(inputs: dict, trace: bool = False):
    teacher = np.ascontiguousarray(np.asarray(inputs["teacher_feat"], dtype=np.float32))
    stu = np.ascontiguousarray(np.asarray(inputs["stu_feat"], dtype=np.float32))
    assert teacher.shape == (B, C, H, W) and stu.shape == (B, C, H, W)
    tch = teacher.reshape(N_CORES, BPC, P, FD)
    sch = stu.reshape(N_CORES, BPC, P, FD)
    in_maps = [
        {"teacher": np.ascontiguousarray(tch[i]), "stu": np.ascontiguousarray(sch[i])}
        for i in range(N_CORES)
    ]
    nc = _get_nc()
    res = run_bass_kernel_spmd(nc, in_maps, core_ids=list(range(N_CORES)), trace=trace)
    parts = [res.results[i]["out"] for i in range(N_CORES)]
    return _combine(parts), res


def kernel(**inputs) -> np.ndarray:
    out, _ = run(inputs, trace=False)
    return np.asarray(out, dtype=np.float32)


if __name__ == "__main__":
    rng = np.random.default_rng(0)
    ins = {
        "teacher_feat": rng.standard_normal((B, C, H, W), dtype=np.float32),
        "stu_feat": rng.standard_normal((B, C, H, W), dtype=np.float32),
    }
    print(kernel(**ins))



# revision 5
# speedup vs baseline: 1.1647x; 1.1647x over previous
"""KD feature-level smooth-L1 loss kernel for Trainium2 (8 NeuronCores).

Math (per batch sample b over (C,H,W) = 256*64*64 = N elements):
  t_norm = (t - mean) * rsqrt(var + eps)          # LayerNorm, no affine
  d   = |t_norm - s|
  kd  = where(d <= 2, d*d/4, d - 1)               # smooth-L1, beta=2
  out = mean_b( sum_chw(kd) )

Device-side decomposition (per sample, rs = 1/sqrt(var+eps), std = 1/rs):
  dt  = |std*s + mean - t|        (= std * d)
  thr = BETA*std
  mt  = min(dt, thr)
  sum(kd) = 0.25*rs^2*sum(mt^2) + rs*sum(dt - mt)
With the two fused accumulator quantities
  C = sum( min(dt,thr) * dt )     (one DVE scalar_tensor_tensor pass)
  E = sum( relu(dt - thr) )       (one ACT pass, bias=-thr)
we have sum(mt^2) = C - thr*E and sum(dt - mt) = E, so
  sum(kd) = 0.25*rs^2*(C - thr*E) + rs*E
Per-sample passes: ACT: Square(t)-stats, Abs(y+mean), Relu(dt-thr);
DVE: y = std*s - t (STT), C-pass (STT); PE: sum(t) via chained matmuls.
Sharding: pure data parallel, 4 samples per core; host combines.
"""

from contextlib import ExitStack

import numpy as np

import concourse.bass as bass
import concourse.mybir as mybir
import concourse.tile as tile
from concourse import bacc
from concourse.bass_utils import run_bass_kernel_spmd

B, C, H, W = 32, 256, 64, 64
N_CORES = 8
BPC = B // N_CORES            # samples per core
P = 128
N = C * H * W                 # 1048576 elements per sample
FD = N // P                   # 8192 free-dim per partition
NCH = 4                       # loss chunks per sample
CH = FD // NCH                # 2048
NSC = 2                       # stats chunks per sample
SC = FD // NSC                # 4096
EPS = 1e-5
BETA = 2.0
LOSS_WEIGHT = 1.0

f32 = mybir.dt.float32
bf16 = mybir.dt.bfloat16
AF = mybir.ActivationFunctionType
OP = mybir.AluOpType
AX = mybir.AxisListType


def _build_kernel(ctx: ExitStack, tc: "tile.TileContext", out_ap, teacher, stu):
    nc = tc.nc

    const_pool = ctx.enter_context(tc.tile_pool(name="const", bufs=1))
    t_pool = ctx.enter_context(tc.tile_pool(name="t", bufs=3))
    s_pool = ctx.enter_context(tc.tile_pool(name="s", bufs=5))
    y_pool = ctx.enter_context(tc.tile_pool(name="y", bufs=2))
    dt_pool = ctx.enter_context(tc.tile_pool(name="dt", bufs=2))
    sq_pool = ctx.enter_context(tc.tile_pool(name="sq", bufs=2))
    cdead_pool = ctx.enter_context(tc.tile_pool(name="cdead", bufs=2))
    edead_pool = ctx.enter_context(tc.tile_pool(name="edead", bufs=2))
    sums_pool = ctx.enter_context(tc.tile_pool(name="sums", bufs=2))
    tiny_pool = ctx.enter_context(tc.tile_pool(name="tiny", bufs=2))
    ps_sumt_pool = ctx.enter_context(tc.tile_pool(name="ps_sumt", bufs=2, space="PSUM"))
    ps_misc_pool = ctx.enter_context(tc.tile_pool(name="ps_misc", bufs=2, space="PSUM"))

    ones = const_pool.tile([P, 1], f32)
    nc.vector.memset(ones[:], 1.0)
    staging = const_pool.tile([1, 16 * BPC], f32)

    for b in range(BPC):
        # ---------------- DMA: teacher halves then student chunks --------
        t_sb = t_pool.tile([P, FD], f32)
        for i in range(NSC):
            nc.sync.dma_start(t_sb[:, i * SC : (i + 1) * SC], teacher[b, :, i * SC : (i + 1) * SC])
        s_tiles = []
        for c in range(NCH):
            s_sb = s_pool.tile([P, CH], f32)
            nc.sync.dma_start(s_sb[:], stu[b, :, c * CH : (c + 1) * CH])
            s_tiles.append(s_sb)

        # cols 0:2 sum(t^2) per stats chunk, 2:6 C per chunk, 6:10 E per chunk
        sums = sums_pool.tile([P, 16], f32)

        # ---------------- stats: S_t (PE), S_tt (ACT Square) -------------
        ps_t = ps_sumt_pool.tile([1, 512], f32)
        nmm = FD // 512
        for k in range(nmm):
            nc.tensor.matmul(
                ps_t[:, :],
                ones[:, :],
                t_sb[:, k * 512 : (k + 1) * 512],
                start=(k == 0),
                stop=(k == nmm - 1),
            )
        for i in range(NSC):
            sq = sq_pool.tile([P, SC], bf16)
            nc.scalar.activation(
                sq[:], t_sb[:, i * SC : (i + 1) * SC], AF.Square,
                accum_out=sums[:, i : i + 1],
            )

        ps_m = ps_misc_pool.tile([1, 16], f32)
        nc.tensor.matmul(ps_m[:, 0:2], ones[:, :], sums[:, 0:2], start=True, stop=True)

        # ---------------- tiny scalar math -------------------------------
        # bb cols: 0=std 1=mean 2=thr 3=-thr 4=S_t 5=S_tt 6..15 scratch
        bb = tiny_pool.tile([1, 16], f32)
        st = bb[0:1, 4:5]
        nc.vector.reduce_sum(out=st, in_=ps_t[:, :], axis=AX.X)
        stt = bb[0:1, 5:6]
        nc.vector.reduce_sum(out=stt, in_=ps_m[0:1, 0:2], axis=AX.X)
        mean = bb[0:1, 1:2]
        nc.vector.tensor_scalar(mean, st, 1.0 / N, None, op0=OP.mult)
        ve_a = bb[0:1, 6:7]
        nc.vector.tensor_scalar(ve_a, stt, 1.0 / N, EPS, op0=OP.mult, op1=OP.add)
        msq = bb[0:1, 7:8]
        nc.vector.tensor_tensor(msq, mean, mean, op=OP.mult)
        ve = bb[0:1, 8:9]
        nc.vector.tensor_tensor(ve, ve_a, msq, op=OP.subtract)
        inv_ve = bb[0:1, 9:10]
        nc.vector.reciprocal(inv_ve, ve)
        rs0 = bb[0:1, 10:11]
        nc.scalar.activation(rs0, inv_ve, AF.Sqrt)  # rs ~= 1/sqrt(ve) (table)
        # one Newton iteration: rs <- rs*(1.5 - 0.5*ve*rs^2)
        r2 = bb[0:1, 11:12]
        nc.vector.tensor_tensor(r2, rs0, rs0, op=OP.mult)
        pv = bb[0:1, 12:13]
        nc.vector.tensor_tensor(pv, r2, ve, op=OP.mult)
        hh = bb[0:1, 13:14]
        nc.vector.tensor_scalar(hh, pv, -0.5, 1.5, op0=OP.mult, op1=OP.add)
        rs = bb[0:1, 14:15]
        nc.vector.tensor_tensor(rs, rs0, hh, op=OP.mult)
        stdv = bb[0:1, 0:1]
        nc.vector.tensor_tensor(stdv, ve, rs, op=OP.mult)  # std = ve*rs = sqrt(ve)
        thr = bb[0:1, 2:3]
        nc.vector.tensor_scalar(thr, stdv, BETA, None, op0=OP.mult)
        nthr = bb[0:1, 3:4]
        nc.vector.tensor_scalar(nthr, stdv, -BETA, None, op0=OP.mult)

        bcast = tiny_pool.tile([P, 4], f32)
        nc.gpsimd.partition_broadcast(bcast[:, 0:4], bb[0:1, 0:4])
        std_vec = bcast[:, 0:1]
        mean_vec = bcast[:, 1:2]
        thr_vec = bcast[:, 2:3]
        nthr_vec = bcast[:, 3:4]

        # ---------------- loss passes ------------------------------------
        for c in range(NCH):
            tsl = slice(c * CH, (c + 1) * CH)
            y = y_pool.tile([P, CH], f32)
            nc.vector.scalar_tensor_tensor(
                y[:], s_tiles[c][:], std_vec, t_sb[:, tsl], op0=OP.mult, op1=OP.subtract
            )
            dt = dt_pool.tile([P, CH], f32)
            nc.scalar.activation(dt[:], y[:], AF.Abs, bias=mean_vec)
            cdead = cdead_pool.tile([P, CH], bf16)
            nc.vector.scalar_tensor_tensor(
                cdead[:], dt[:], thr_vec, dt[:], op0=OP.min, op1=OP.mult,
                accum_out=sums[:, 2 + c : 3 + c],
            )
            edead = edead_pool.tile([P, CH], bf16)
            nc.scalar.activation(
                edead[:], dt[:], AF.Relu, bias=nthr_vec,
                accum_out=sums[:, 6 + c : 7 + c],
            )

        # partition-reduce the 8 loss partials in one matmul
        nc.tensor.matmul(ps_m[:, 2:10], ones[:, :], sums[:, 2:10], start=True, stop=True)
        nc.vector.tensor_copy(staging[0:1, 16 * b : 16 * b + 2], ps_m[0:1, 0:2])
        nc.vector.tensor_copy(staging[0:1, 16 * b + 2 : 16 * b + 10], ps_m[0:1, 2:10])
        nc.vector.tensor_copy(staging[0:1, 16 * b + 10 : 16 * b + 11], rs)
        nc.vector.tensor_copy(staging[0:1, 16 * b + 11 : 16 * b + 12], stdv)
        nc.vector.tensor_copy(staging[0:1, 16 * b + 12 : 16 * b + 13], bb[0:1, 1:2])
        nc.vector.tensor_copy(staging[0:1, 16 * b + 13 : 16 * b + 14], thr)
        nc.vector.tensor_copy(staging[0:1, 16 * b + 14 : 16 * b + 15], ve)
        nc.vector.tensor_copy(staging[0:1, 16 * b + 15 : 16 * b + 16], st)
        nc.sync.dma_start(out_ap[:, 16 * b : 16 * b + 16], staging[:, 16 * b : 16 * b + 16])


_CACHED = {}


def _get_nc():
    if "nc" in _CACHED:
        return _CACHED["nc"]
    nc = bacc.Bacc(
        "TRN2",
        target_bir_lowering=False,
        debug=False,
        enable_asserts=False,
        num_devices=N_CORES,
    )
    teacher = nc.dram_tensor("teacher", [BPC, P, FD], f32, kind="ExternalInput").ap()
    stu = nc.dram_tensor("stu", [BPC, P, FD], f32, kind="ExternalInput").ap()
    out = nc.dram_tensor("out", [1, 16 * BPC], f32, kind="ExternalOutput").ap()
    with tile.TileContext(nc) as tc:
        with ExitStack() as ctx:
            _build_kernel(ctx, tc, out, teacher, stu)
    nc.compile()
    _CACHED["nc"] = nc
    return nc


def _combine(parts):
    """parts: list of 8 arrays [1, 16*BPC] -> scalar loss (float64 math)."""
    losses = []
    for r in parts:
        r = np.asarray(r, dtype=np.float64).reshape(BPC, 16)
        Cs = r[:, 2:6].sum(axis=1)
        Es = r[:, 6:10].sum(axis=1)
        rs = r[:, 10]
        thr = r[:, 13]
        losses.append(0.25 * rs * rs * (Cs - thr * Es) + rs * Es)
    losses = np.concatenate(losses)
    return np.float32(LOSS_WEIGHT * losses.mean())


def run(inputs: dict, trace: bool = False):
    teacher = np.ascontiguousarray(np.asarray(inputs["teacher_feat"], dtype=np.float32))
    stu = np.ascontiguousarray(np.asarray(inputs["stu_feat"], dtype=np.float32))
    assert teacher.shape == (B, C, H, W) and stu.shape == (B, C, H, W)
    tch = teacher.reshape(N_CORES, BPC, P, FD)
    sch = stu.reshape(N_CORES, BPC, P, FD)
    in_maps = [
        {"teacher": np.ascontiguousarray(tch[i]), "stu": np.ascontiguousarray(sch[i])}
        for i in range(N_CORES)
    ]
    nc = _get_nc()
    res = run_bass_kernel_spmd(nc, in_maps, core_ids=list(range(N_CORES)), trace=trace)
    parts = [res.results[i]["out"] for i in range(N_CORES)]
    return _combine(parts), res


def kernel(**inputs) -> np.ndarray:
    out, _ = run(inputs, trace=False)
    return np.asarray(out, dtype=np.float32)


if __name__ == "__main__":
    rng = np.random.default_rng(0)
    ins = {
        "teacher_feat": rng.standard_normal((B, C, H, W), dtype=np.float32),
        "stu_feat": rng.standard_normal((B, C, H, W), dtype=np.float32),
    }
    print(kernel(**ins))


# revision 14
# speedup vs baseline: 1.2349x; 1.0603x over previous
"""KD feature-level smooth-L1 loss kernel for Trainium2 (8 NeuronCores).

Math (per batch sample b over (C,H,W) = 256*64*64 = N elements):
  t_norm = (t - mean) * rsqrt(var + eps)          # LayerNorm, no affine
  d   = |t_norm - s|
  kd  = where(d <= 2, d*d/4, d - 1)               # smooth-L1, beta=2
  out = mean_b( sum_chw(kd) )

Device-side decomposition, computed directly in normalized space
(rs = 1/sqrt(var+eps)):
  x  = (s + rs*mean) - rs*t                       # = s - t_norm = -(t_norm - s)
  Q  = sum( min(x^2, BETA^2) )
  E  = sum( relu(|x| - BETA) )
  sum(kd) = 0.25*Q + E
Q and E are each ONE fused custom DVE op (affine-combine + clamp/abs +
accumulate), so the whole loss phase is 2 DVE passes per element pair.
Stats use ACT: Identity+accum (sum t) and Square+accum (sum t^2).
Sharding: pure data parallel, 4 samples per core; host combines.
"""

import re
from contextlib import ExitStack

import numpy as np

import concourse.bass as bass
import concourse.mybir as mybir
import concourse.tile as tile
from concourse import bacc, dve_ops
from concourse.bass_utils import run_bass_kernel_spmd
from concourse.dve_spec import (
    Spec, Src0, Src1, C0, C1, C2, Zero, relu, sq, maxx, minn, AluOp as DveAluOp,
)

B, C, H, W = 32, 256, 64, 64
N_CORES = 8
BPC = B // N_CORES            # samples per core
P = 128
N = C * H * W                 # 1048576 elements per sample
FD = N // P                   # 8192 free-dim per partition
NCH = 4                       # loss chunks per sample
CH = FD // NCH                # 2048
NSC = 2                       # stats chunks per sample
SC = FD // NSC                # 4096
EPS = 1e-5
BETA = 2.0
LOSS_WEIGHT = 1.0

f32 = mybir.dt.float32
bf16 = mybir.dt.bfloat16
AF = mybir.ActivationFunctionType
OP = mybir.AluOpType
AX = mybir.AxisListType


# --------------- custom fused DVE ops (Q and E passes) -----------------------
def _register_dve_op(name: str, spec: "Spec") -> "dve_ops.DveOp":
    for existing in dve_ops.OPS:
        if existing.name == name:
            return existing
    op = dve_ops.DveOp(name, spec, subdim=False, uops_sha={})
    dve_ops._SUB_OPCODE_FOR_NAME[name] = max(dve_ops._SUB_OPCODE_FOR_NAME.values()) + 1
    try:
        op.compile("v3")
    except ValueError as e:
        m = re.search(r"v3: ([0-9a-f]+)", str(e))
        if not m:
            raise
        op.uops_sha["v3"] = m.group(1)
    op.compile("v3")
    dve_ops.OPS.append(op)
    dve_ops.CUSTOM_DVE_SPECS[name] = spec
    return op


def _q_ref(in0, in1, s0, s1, imm2):
    x = (in0.astype(np.float32) + s1) - in1.astype(np.float32) * s0
    return np.minimum(x * x, imm2)


def _e_ref(in0, in1, s0, s1, imm2):
    x = (in0.astype(np.float32) + s1) - in1.astype(np.float32) * s0
    return np.maximum(np.abs(x) - imm2, 0.0)


_xq = (Src0 + C1) - Src1 * C0
Q_OP = _register_dve_op(
    "KD_SL1_Q_ANT",
    Spec(body=minn(sq(_xq), C2), accum=DveAluOp.ADD,
         reference=dve_ops._ref_body_sum(_q_ref)),
)
_xe = (Src0 + C1) - Src1 * C0
E_OP = _register_dve_op(
    "KD_SL1_E_ANT",
    Spec(body=relu(maxx(_xe, Zero - _xe) - C2), accum=DveAluOp.ADD,
         reference=dve_ops._ref_body_sum(_e_ref)),
)


def _build_kernel(ctx: ExitStack, tc: "tile.TileContext", out_ap, teacher, stu):
    nc = tc.nc

    const_pool = ctx.enter_context(tc.tile_pool(name="const", bufs=1))
    t_pool = ctx.enter_context(tc.tile_pool(name="t", bufs=3))
    s_pool = ctx.enter_context(tc.tile_pool(name="s", bufs=6))
    id_pool = ctx.enter_context(tc.tile_pool(name="iddead", bufs=2))
    sq_pool = ctx.enter_context(tc.tile_pool(name="sqdead", bufs=2))
    qdead_pool = ctx.enter_context(tc.tile_pool(name="qdead", bufs=2))
    edead_pool = ctx.enter_context(tc.tile_pool(name="edead", bufs=2))
    sums_pool = ctx.enter_context(tc.tile_pool(name="sums", bufs=2))
    tiny_pool = ctx.enter_context(tc.tile_pool(name="tiny", bufs=2))
    ps_misc_pool = ctx.enter_context(tc.tile_pool(name="ps_misc", bufs=2, space="PSUM"))

    onesf = const_pool.tile([P, 1], f32)
    nc.vector.memset(onesf[:], 1.0)
    staging = const_pool.tile([1, 16 * BPC], f32)

    for b in range(BPC):
        # ---------------- DMA: teacher halves then student chunks --------
        t_sb = t_pool.tile([P, FD], f32)
        for i in range(NSC):
            nc.sync.dma_start(t_sb[:, i * SC : (i + 1) * SC], teacher[b, :, i * SC : (i + 1) * SC])
        s_tiles = []
        for c in range(NCH):
            s_sb = s_pool.tile([P, CH], f32)
            nc.sync.dma_start(s_sb[:], stu[b, :, c * CH : (c + 1) * CH])
            s_tiles.append(s_sb)

        # cols 0:2 sum(t) per stats chunk, 2:4 sum(t^2), 4:8 Q, 8:12 E
        sums = sums_pool.tile([P, 16], f32)

        # ---------------- stats: S_t (ACT Identity), S_tt (ACT Square) ---
        for i in range(NSC):
            sl = slice(i * SC, (i + 1) * SC)
            iddead = id_pool.tile([P, SC], bf16)
            nc.scalar.activation(iddead[:], t_sb[:, sl], AF.Identity,
                                 accum_out=sums[:, i : i + 1])
            sqdead = sq_pool.tile([P, SC], bf16)
            nc.scalar.activation(sqdead[:], t_sb[:, sl], AF.Square,
                                 accum_out=sums[:, 2 + i : 3 + i])

        ps_m = ps_misc_pool.tile([1, 16], f32)
        nc.tensor.matmul(ps_m[:, 0:4], onesf[:, :], sums[:, 0:4], start=True, stop=True)

        # ---------------- tiny scalar math -------------------------------
        # bb cols: 0=rs 1=rs*mean 2=S_t 3=S_tt 4.. scratch
        bb = tiny_pool.tile([1, 16], f32)
        st = bb[0:1, 2:3]
        nc.vector.reduce_sum(out=st, in_=ps_m[0:1, 0:2], axis=AX.X)
        stt = bb[0:1, 3:4]
        nc.vector.reduce_sum(out=stt, in_=ps_m[0:1, 2:4], axis=AX.X)
        mean = bb[0:1, 4:5]
        nc.vector.tensor_scalar(mean, st, 1.0 / N, None, op0=OP.mult)
        ve_a = bb[0:1, 5:6]
        nc.vector.tensor_scalar(ve_a, stt, 1.0 / N, EPS, op0=OP.mult, op1=OP.add)
        msq = bb[0:1, 6:7]
        nc.vector.tensor_tensor(msq, mean, mean, op=OP.mult)
        ve = bb[0:1, 7:8]
        nc.vector.tensor_tensor(ve, ve_a, msq, op=OP.subtract)
        inv_ve = bb[0:1, 8:9]
        nc.vector.reciprocal(inv_ve, ve)
        rs0 = bb[0:1, 9:10]
        nc.scalar.activation(rs0, inv_ve, AF.Sqrt)  # rs ~= 1/sqrt(ve) (table)
        # one Newton iteration: rs <- rs*(1.5 - 0.5*ve*rs^2)
        r2 = bb[0:1, 10:11]
        nc.vector.tensor_tensor(r2, rs0, rs0, op=OP.mult)
        pv = bb[0:1, 11:12]
        nc.vector.tensor_tensor(pv, r2, ve, op=OP.mult)
        hh = bb[0:1, 12:13]
        nc.vector.tensor_scalar(hh, pv, -0.5, 1.5, op0=OP.mult, op1=OP.add)
        rs = bb[0:1, 0:1]
        nc.vector.tensor_tensor(rs, rs0, hh, op=OP.mult)
        rsm = bb[0:1, 1:2]
        nc.vector.tensor_tensor(rsm, rs, mean, op=OP.mult)

        bcast = tiny_pool.tile([P, 2], f32)
        nc.gpsimd.partition_broadcast(bcast[:, 0:2], bb[0:1, 0:2])
        rs_vec = bcast[:, 0:1]
        rsm_vec = bcast[:, 1:2]

        # ---------------- loss passes: fused Q and E ---------------------
        for c in range(NCH):
            tsl = slice(c * CH, (c + 1) * CH)
            qdead = qdead_pool.tile([P, CH], bf16)
            nc.vector._custom_dve(
                Q_OP, out=qdead[:], accum_out=sums[:, 4 + c : 5 + c],
                in0=s_tiles[c][:], in1=t_sb[:, tsl], s0=rs_vec, s1=rsm_vec,
                imm2=BETA * BETA,
            )
            edead = edead_pool.tile([P, CH], bf16)
            nc.vector._custom_dve(
                E_OP, out=edead[:], accum_out=sums[:, 8 + c : 9 + c],
                in0=s_tiles[c][:], in1=t_sb[:, tsl], s0=rs_vec, s1=rsm_vec,
                imm2=BETA,
            )

        # partition-reduce the 8 loss partials in one matmul
        nc.tensor.matmul(ps_m[:, 4:12], onesf[:, :], sums[:, 4:12], start=True, stop=True)
        nc.vector.tensor_copy(staging[0:1, 16 * b : 16 * b + 4], ps_m[0:1, 0:4])
        nc.vector.tensor_copy(staging[0:1, 16 * b + 4 : 16 * b + 12], ps_m[0:1, 4:12])
        nc.vector.tensor_copy(staging[0:1, 16 * b + 12 : 16 * b + 13], rs)
        nc.vector.tensor_copy(staging[0:1, 16 * b + 13 : 16 * b + 14], rsm)
        nc.vector.tensor_copy(staging[0:1, 16 * b + 14 : 16 * b + 15], ve)
        nc.vector.tensor_copy(staging[0:1, 16 * b + 15 : 16 * b + 16], mean)
        nc.sync.dma_start(out_ap[:, 16 * b : 16 * b + 16], staging[:, 16 * b : 16 * b + 16])


_CACHED = {}


def _get_nc():
    if "nc" in _CACHED:
        return _CACHED["nc"]
    nc = bacc.Bacc(
        "TRN2",
        target_bir_lowering=False,
        debug=False,
        enable_asserts=False,
        num_devices=N_CORES,
    )
    teacher = nc.dram_tensor("teacher", [BPC, P, FD], f32, kind="ExternalInput").ap()
    stu = nc.dram_tensor("stu", [BPC, P, FD], f32, kind="ExternalInput").ap()
    out = nc.dram_tensor("out", [1, 16 * BPC], f32, kind="ExternalOutput").ap()
    with tile.TileContext(nc) as tc:
        with ExitStack() as ctx:
            _build_kernel(ctx, tc, out, teacher, stu)
    nc.compile()
    _CACHED["nc"] = nc
    return nc


def _combine(parts):
    """parts: list of 8 arrays [1, 16*BPC] -> scalar loss (float64 math)."""
    losses = []
    for r in parts:
        r = np.asarray(r, dtype=np.float64).reshape(BPC, 16)
        Q = r[:, 4:8].sum(axis=1)
        E = r[:, 8:12].sum(axis=1)
        losses.append(0.25 * Q + E)
    losses = np.concatenate(losses)
    return np.float32(LOSS_WEIGHT * losses.mean())


def run(inputs: dict, trace: bool = False):
    teacher = np.ascontiguousarray(np.asarray(inputs["teacher_feat"], dtype=np.float32))
    stu = np.ascontiguousarray(np.asarray(inputs["stu_feat"], dtype=np.float32))
    assert teacher.shape == (B, C, H, W) and stu.shape == (B, C, H, W)
    tch = teacher.reshape(N_CORES, BPC, P, FD)
    sch = stu.reshape(N_CORES, BPC, P, FD)
    in_maps = [
        {"teacher": np.ascontiguousarray(tch[i]), "stu": np.ascontiguousarray(sch[i])}
        for i in range(N_CORES)
    ]
    nc = _get_nc()
    res = run_bass_kernel_spmd(nc, in_maps, core_ids=list(range(N_CORES)), trace=trace)
    parts = [res.results[i]["out"] for i in range(N_CORES)]
    return _combine(parts), res


def kernel(**inputs) -> np.ndarray:
    out, _ = run(inputs, trace=False)
    return np.asarray(out, dtype=np.float32)


if __name__ == "__main__":
    rng = np.random.default_rng(0)
    ins = {
        "teacher_feat": rng.standard_normal((B, C, H, W), dtype=np.float32),
        "stu_feat": rng.standard_normal((B, C, H, W), dtype=np.float32),
    }
    print(kernel(**ins))
